# revision 5
# baseline (speedup 1.0000x reference)
"""ChebyKAN layer kernel for 8x Trainium2 NeuronCores.

Computes y[b,o] = sum_{i,d} T_d(tanh(x[b,i])) * C[i,o,d], d = 0..8,
with T_d the Chebyshev polynomials, via:
  - batch sharded 8 ways (1024 rows/core)
  - device computes T_1..T_8 with Chebyshev product identities
    (fp32 DVE/ACT), casts basis to bf16
  - d=0 term (T_0 == 1) folded into a host-precomputed bias[o]
  - big contraction as bf16 matmuls accumulating fp32 in PSUM:
    K = (i,d) of size 8192 in 64 chunks of 128
  - x is transposed on host so the basis is produced directly in
    [K, batch] (lhsT) layout; no on-device transpose needed.

Self-contained: hardcodes all shapes for inputs
  x: [8192, 1024] f32, cheby_coeffs: [1024, 1024, 9] f32.
"""

import numpy as np
import ml_dtypes

import concourse.bass as bass
import concourse.mybir as mybir
import concourse.tile as tile
from concourse import bacc
from concourse.bass_utils import run_bass_kernel_spmd

P = 128
B_TOTAL = 8192
I_DIM = 1024
O_DIM = 1024
DEG = 8              # degrees 1..8 on device (d=0 folded into bias)
N_CORES = 8
B_LOCAL = B_TOTAL // N_CORES     # 1024
IC = I_DIM // P                  # 8 input chunks
NK = IC * DEG                    # 64 K-chunks of 128
OH = 2                           # output halves (PSUM capacity: 8 banks)
ON = O_DIM // OH                 # 512

_nc = None
last_results = None  # BassKernelResults of the most recent run (for profiling)


def _build_nc():
    nc = bacc.Bacc()
    f32 = mybir.dt.float32
    bf16 = mybir.dt.bfloat16
    AF = mybir.ActivationFunctionType
    ALU = mybir.AluOpType

    xt_d = nc.dram_tensor("xt", [I_DIM, B_LOCAL], f32, kind="ExternalInput")
    w_d = nc.dram_tensor("w", [OH, NK, P, ON], bf16, kind="ExternalInput")
    bias_d = nc.dram_tensor("bias", [P, O_DIM], f32, kind="ExternalInput")
    y_d = nc.dram_tensor("y", [B_LOCAL, O_DIM], f32, kind="ExternalOutput")

    with tile.TileContext(nc) as tc:
        with (
            tc.tile_pool(name="const", bufs=1) as cpool,
            tc.tile_pool(name="xin", bufs=2) as xpool,
            tc.tile_pool(name="fwork", bufs=2) as fpool,
            tc.tile_pool(name="basis", bufs=1) as bpool,
            tc.tile_pool(name="wstream", bufs=4) as wpool,
            tc.tile_pool(name="outbuf", bufs=4) as opool,
            tc.tile_pool(name="acc", bufs=1, space="PSUM") as ppool,
        ):
            bias_t = cpool.tile([P, O_DIM], f32, name="bias_t")
            nc.gpsimd.dma_start(out=bias_t, in_=bias_d[:, :])

            # ---- basis production: T_1..T_8 per 128-row chunk of i ----
            basis = {}

            def emit_cast(ic, d, src, use_dve):
                bt = bpool.tile([P, B_LOCAL], bf16, tag=f"b_{ic}_{d}",
                                name=f"b_{ic}_{d}")
                if use_dve:
                    nc.vector.tensor_copy(bt, src)
                else:
                    nc.scalar.copy(bt, src)
                basis[(ic, d)] = bt

            for ic in range(IC):
                xt_t = xpool.tile([P, B_LOCAL], f32, tag="xt", name=f"xt_{ic}")
                nc.gpsimd.dma_start(out=xt_t, in_=xt_d[ic * P:(ic + 1) * P, :])

                # T1 = tanh(x) (no clip: the recurrence is stable for |t|<=1
                # and T_d(+-1) is finite; deviation from the reference's
                # clip at 0.999 is ~1e-6 on y)
                t = fpool.tile([P, B_LOCAL], f32, tag="T1", name=f"t_{ic}")
                nc.scalar.activation(t, xt_t, AF.Tanh)
                emit_cast(ic, 1, t, use_dve=False)

                # T2 = 2 t^2 - 1
                s2 = fpool.tile([P, B_LOCAL], f32, tag="sq", name=f"s2_{ic}")
                nc.scalar.square(s2, t)
                T2 = fpool.tile([P, B_LOCAL], f32, tag="T2", name=f"T2_{ic}",
                                bufs=1)
                nc.vector.tensor_scalar(T2, s2, 2.0, -1.0, ALU.mult, ALU.add)
                emit_cast(ic, 2, T2, use_dve=False)

                # T3 = 2 t T2 - t
                u3 = fpool.tile([P, B_LOCAL], f32, tag="u", name=f"u3_{ic}")
                nc.vector.tensor_mul(u3, t, T2)
                T3 = fpool.tile([P, B_LOCAL], f32, tag="T3", name=f"T3_{ic}",
                                bufs=1)
                nc.vector.scalar_tensor_tensor(T3, u3, 2.0, t,
                                               ALU.mult, ALU.subtract)
                emit_cast(ic, 3, T3, use_dve=False)

                # T4 = 2 T2^2 - 1
                s4 = fpool.tile([P, B_LOCAL], f32, tag="sq", name=f"s4_{ic}")
                nc.scalar.square(s4, T2)
                T4 = fpool.tile([P, B_LOCAL], f32, tag="T4", name=f"T4_{ic}",
                                bufs=1)
                nc.vector.tensor_scalar(T4, s4, 2.0, -1.0, ALU.mult, ALU.add)
                emit_cast(ic, 4, T4, use_dve=False)

                # T5 = 2 T2 T3 - t
                u5 = fpool.tile([P, B_LOCAL], f32, tag="u", name=f"u5_{ic}")
                nc.vector.tensor_mul(u5, T2, T3)
                T5 = fpool.tile([P, B_LOCAL], f32, tag="Ttmp", name=f"T5_{ic}")
                nc.vector.scalar_tensor_tensor(T5, u5, 2.0, t,
                                               ALU.mult, ALU.subtract)
                emit_cast(ic, 5, T5, use_dve=True)

                # T6 = 2 T3^2 - 1
                s6 = fpool.tile([P, B_LOCAL], f32, tag="sq", name=f"s6_{ic}")
                nc.scalar.square(s6, T3)
                T6 = fpool.tile([P, B_LOCAL], f32, tag="Ttmp", name=f"T6_{ic}")
                nc.vector.tensor_scalar(T6, s6, 2.0, -1.0, ALU.mult, ALU.add)
                emit_cast(ic, 6, T6, use_dve=False)

                # T7 = 2 T3 T4 - t
                u7 = fpool.tile([P, B_LOCAL], f32, tag="u", name=f"u7_{ic}")
                nc.vector.tensor_mul(u7, T3, T4)
                T7 = fpool.tile([P, B_LOCAL], f32, tag="Ttmp", name=f"T7_{ic}")
                nc.vector.scalar_tensor_tensor(T7, u7, 2.0, t,
                                               ALU.mult, ALU.subtract)
                emit_cast(ic, 7, T7, use_dve=True)

                # T8 = 2 T4^2 - 1
                s8 = fpool.tile([P, B_LOCAL], f32, tag="sq", name=f"s8_{ic}")
                nc.scalar.square(s8, T4)
                T8 = fpool.tile([P, B_LOCAL], f32, tag="Ttmp", name=f"T8_{ic}")
                nc.vector.tensor_scalar(T8, s8, 2.0, -1.0, ALU.mult, ALU.add)
                emit_cast(ic, 8, T8, use_dve=False)

            # ---- contraction: two o-half passes over all 64 K-chunks ----
            psums = [ppool.tile([P, ON], f32, tag=f"ps{b}", name=f"ps{b}")
                     for b in range(B_LOCAL // P)]
            for oh in range(OH):
                for k in range(NK):
                    ic, dm1 = divmod(k, DEG)
                    wt = wpool.tile([P, ON], bf16, tag="wt",
                                    name=f"wt_{oh}_{k}")
                    nc.gpsimd.dma_start(out=wt, in_=w_d[oh, k])
                    bt = basis[(ic, dm1 + 1)]
                    for b in range(B_LOCAL // P):
                        nc.tensor.matmul(
                            psums[b],
                            bt[:, b * P:(b + 1) * P],
                            wt,
                            start=(k == 0),
                            stop=(k == NK - 1),
                        )
                for b in range(B_LOCAL // P):
                    ot = opool.tile([P, ON], f32, tag="ot", name=f"ot_{oh}_{b}")
                    nc.vector.tensor_add(ot, psums[b],
                                         bias_t[:, oh * ON:(oh + 1) * ON])
                    nc.gpsimd.dma_start(
                        out=y_d[b * P:(b + 1) * P, oh * ON:(oh + 1) * ON],
                        in_=ot)
    nc.compile()  # bacc legalization: splits multi-sem waits (TRN2 allows 1)
    return nc


def _get_nc():
    global _nc
    if _nc is None:
        _nc = _build_nc()
    return _nc


def _prep_inputs(x, cheby_coeffs):
    x = np.asarray(x, dtype=np.float32)
    C = np.asarray(cheby_coeffs, dtype=np.float32)
    bf16 = ml_dtypes.bfloat16

    # W[oh, k=(ic,d), p, on] = C[ic*128+p, oh*512+on, d+1]
    Wd = C[:, :, 1:]                                   # [I, O, 8]
    Wd = Wd.reshape(IC, P, OH, ON, DEG)                # [ic, p, oh, on, d]
    Wd = np.transpose(Wd, (2, 0, 4, 1, 3))             # [oh, ic, d, p, on]
    Wd = np.ascontiguousarray(Wd.reshape(OH, NK, P, ON)).astype(bf16)

    bias = C[:, :, 0].sum(axis=0, dtype=np.float64).astype(np.float32)
    bias_rep = np.ascontiguousarray(np.broadcast_to(bias, (P, O_DIM)))

    in_maps = []
    for c in range(N_CORES):
        xt = np.ascontiguousarray(x[c * B_LOCAL:(c + 1) * B_LOCAL, :].T)
        in_maps.append({"xt": xt, "w": Wd, "bias": bias_rep})
    return in_maps


def kernel(x, cheby_coeffs):
    global last_results
    nc = _get_nc()
    in_maps = _prep_inputs(x, cheby_coeffs)
    last_results = run_bass_kernel_spmd(nc, in_maps,
                                        core_ids=list(range(N_CORES)))
    y = np.concatenate([r["y"] for r in last_results.results], axis=0)
    return y


# revision 10
# speedup vs baseline: 1.0919x; 1.0919x over previous
"""ChebyKAN layer kernel for 8x Trainium2 NeuronCores.

Computes y[b,o] = sum_{i,d} T_d(tanh(x[b,i])) * C[i,o,d], d = 0..8,
with T_d the Chebyshev polynomials, via:
  - batch sharded 8 ways (1024 rows/core)
  - device computes T_1..T_8 with Chebyshev product identities
    (fp32 DVE/ACT), casts basis to bf16
  - d=0 term (T_0 == 1) folded into a host-precomputed bias[o]
  - big contraction as bf16 matmuls accumulating fp32 in PSUM:
    K = (i,d) of size 8192 in 64 chunks of 128
  - x is transposed on host so the basis is produced directly in
    [K, batch] (lhsT) layout; no on-device transpose needed.

Self-contained: hardcodes all shapes for inputs
  x: [8192, 1024] f32, cheby_coeffs: [1024, 1024, 9] f32.
"""

import numpy as np
import ml_dtypes

import concourse.bass as bass
import concourse.mybir as mybir
import concourse.tile as tile
from concourse import bacc
from concourse.bass_utils import run_bass_kernel_spmd

P = 128
B_TOTAL = 8192
I_DIM = 1024
O_DIM = 1024
DEG = 8              # degrees 1..8 on device (d=0 folded into bias)
N_CORES = 8
B_LOCAL = B_TOTAL // N_CORES     # 1024
IC = I_DIM // P                  # 8 input chunks
NK = IC * DEG                    # 64 K-chunks of 128
OH = 2                           # output halves (PSUM capacity: 8 banks)
ON = O_DIM // OH                 # 512

_nc = None
last_results = None  # BassKernelResults of the most recent run (for profiling)


def _build_nc():
    nc = bacc.Bacc()
    f32 = mybir.dt.float32
    bf16 = mybir.dt.bfloat16
    AF = mybir.ActivationFunctionType
    ALU = mybir.AluOpType

    xt_d = nc.dram_tensor("xt", [I_DIM, B_LOCAL], f32, kind="ExternalInput")
    w_d = nc.dram_tensor("w", [OH, NK, P, ON], bf16, kind="ExternalInput")
    bias_d = nc.dram_tensor("bias", [P, O_DIM], f32, kind="ExternalInput")
    y_d = nc.dram_tensor("y", [B_LOCAL, O_DIM], f32, kind="ExternalOutput")

    with tile.TileContext(nc) as tc:
        with (
            tc.tile_pool(name="const", bufs=1) as cpool,
            tc.tile_pool(name="xin", bufs=2) as xpool,
            tc.tile_pool(name="fwork", bufs=2) as fpool,
            tc.tile_pool(name="basis", bufs=1) as bpool,
            tc.tile_pool(name="wstream", bufs=4) as wpool,
            tc.tile_pool(name="outbuf", bufs=4) as opool,
            tc.tile_pool(name="acc", bufs=1, space="PSUM") as ppool,
        ):
            # ---- basis production: T_1..T_8 per 128-row chunk of i ----
            basis = {}

            def emit_cast(ic, d, src, use_dve):
                bt = bpool.tile([P, B_LOCAL], bf16, tag=f"b_{ic}_{d}",
                                name=f"b_{ic}_{d}")
                if use_dve:
                    nc.vector.tensor_copy(bt, src)
                else:
                    nc.scalar.copy(bt, src)
                basis[(ic, d)] = bt

            for ic in range(IC):
                # xt on the HWDGE (sync) queue: issues in parallel with the
                # gpsimd wt stream and has lower first-byte latency, so the
                # first tanh can start ~5us sooner.
                xt_t = xpool.tile([P, B_LOCAL], f32, tag="xt", name=f"xt_{ic}")
                nc.sync.dma_start(out=xt_t, in_=xt_d[ic * P:(ic + 1) * P, :])

                # T1 = tanh(x) (no clip: the recurrence is stable for |t|<=1
                # and T_d(+-1) is finite; deviation from the reference's
                # clip at 0.999 is ~1e-6 on y)
                t = fpool.tile([P, B_LOCAL], f32, tag="T1", name=f"t_{ic}")
                nc.scalar.activation(t, xt_t, AF.Tanh)
                # DVE cast: shortens the tanh -> first-matmul critical chain
                emit_cast(ic, 1, t, use_dve=True)

                # T2 = 2 t^2 - 1
                s2 = fpool.tile([P, B_LOCAL], f32, tag="sq", name=f"s2_{ic}")
                nc.scalar.square(s2, t)
                T2 = fpool.tile([P, B_LOCAL], f32, tag="T2", name=f"T2_{ic}",
                                bufs=1)
                nc.vector.tensor_scalar(T2, s2, 2.0, -1.0, ALU.mult, ALU.add)
                emit_cast(ic, 2, T2, use_dve=False)

                # T3 = 2 t T2 - t
                u3 = fpool.tile([P, B_LOCAL], f32, tag="u", name=f"u3_{ic}")
                nc.vector.tensor_mul(u3, t, T2)
                T3 = fpool.tile([P, B_LOCAL], f32, tag="T3", name=f"T3_{ic}",
                                bufs=1)
                nc.vector.scalar_tensor_tensor(T3, u3, 2.0, t,
                                               ALU.mult, ALU.subtract)
                emit_cast(ic, 3, T3, use_dve=False)

                # T4 = 2 T2^2 - 1
                s4 = fpool.tile([P, B_LOCAL], f32, tag="sq", name=f"s4_{ic}")
                nc.scalar.square(s4, T2)
                T4 = fpool.tile([P, B_LOCAL], f32, tag="T4", name=f"T4_{ic}",
                                bufs=1)
                nc.vector.tensor_scalar(T4, s4, 2.0, -1.0, ALU.mult, ALU.add)
                emit_cast(ic, 4, T4, use_dve=False)

                # T5 = 2 T2 T3 - t
                u5 = fpool.tile([P, B_LOCAL], f32, tag="u", name=f"u5_{ic}")
                nc.vector.tensor_mul(u5, T2, T3)
                T5 = fpool.tile([P, B_LOCAL], f32, tag="Ttmp", name=f"T5_{ic}")
                nc.vector.scalar_tensor_tensor(T5, u5, 2.0, t,
                                               ALU.mult, ALU.subtract)
                emit_cast(ic, 5, T5, use_dve=True)

                # T6 = 2 T3^2 - 1
                s6 = fpool.tile([P, B_LOCAL], f32, tag="sq", name=f"s6_{ic}")
                nc.scalar.square(s6, T3)
                T6 = fpool.tile([P, B_LOCAL], f32, tag="Ttmp", name=f"T6_{ic}")
                nc.vector.tensor_scalar(T6, s6, 2.0, -1.0, ALU.mult, ALU.add)
                emit_cast(ic, 6, T6, use_dve=False)

                # T7 = 2 T3 T4 - t
                u7 = fpool.tile([P, B_LOCAL], f32, tag="u", name=f"u7_{ic}")
                nc.vector.tensor_mul(u7, T3, T4)
                T7 = fpool.tile([P, B_LOCAL], f32, tag="Ttmp", name=f"T7_{ic}")
                nc.vector.scalar_tensor_tensor(T7, u7, 2.0, t,
                                               ALU.mult, ALU.subtract)
                emit_cast(ic, 7, T7, use_dve=True)

                # T8 = 2 T4^2 - 1
                s8 = fpool.tile([P, B_LOCAL], f32, tag="sq", name=f"s8_{ic}")
                nc.scalar.square(s8, T4)
                T8 = fpool.tile([P, B_LOCAL], f32, tag="Ttmp", name=f"T8_{ic}")
                nc.vector.tensor_scalar(T8, s8, 2.0, -1.0, ALU.mult, ALU.add)
                emit_cast(ic, 8, T8, use_dve=False)

            # bias is only consumed at the end of each o-half pass; load it
            # late so it doesn't delay the xt/wt streams.
            bias_t = cpool.tile([P, O_DIM], f32, name="bias_t")
            nc.sync.dma_start(out=bias_t, in_=bias_d[:, :])

            # ---- contraction: two o-half passes over all 64 K-chunks ----
            psums = [ppool.tile([P, ON], f32, tag=f"ps{b}", name=f"ps{b}")
                     for b in range(B_LOCAL // P)]
            for oh in range(OH):
                for k in range(NK):
                    ic, dm1 = divmod(k, DEG)
                    wt = wpool.tile([P, ON], bf16, tag="wt",
                                    name=f"wt_{oh}_{k}")
                    nc.gpsimd.dma_start(out=wt, in_=w_d[oh, k])
                    bt = basis[(ic, dm1 + 1)]
                    for b in range(B_LOCAL // P):
                        nc.tensor.matmul(
                            psums[b],
                            bt[:, b * P:(b + 1) * P],
                            wt,
                            start=(k == 0),
                            stop=(k == NK - 1),
                        )
                for b in range(B_LOCAL // P):
                    ot = opool.tile([P, ON], f32, tag="ot", name=f"ot_{oh}_{b}")
                    nc.vector.tensor_add(ot, psums[b],
                                         bias_t[:, oh * ON:(oh + 1) * ON])
                    nc.sync.dma_start(
                        out=y_d[b * P:(b + 1) * P, oh * ON:(oh + 1) * ON],
                        in_=ot)
    nc.compile()  # bacc legalization: splits multi-sem waits (TRN2 allows 1)
    return nc


def _get_nc():
    global _nc
    if _nc is None:
        _nc = _build_nc()
    return _nc


def _prep_inputs(x, cheby_coeffs):
    x = np.asarray(x, dtype=np.float32)
    C = np.asarray(cheby_coeffs, dtype=np.float32)
    bf16 = ml_dtypes.bfloat16

    # W[oh, k=(ic,d), p, on] = C[ic*128+p, oh*512+on, d+1]
    Wd = C[:, :, 1:]                                   # [I, O, 8]
    Wd = Wd.reshape(IC, P, OH, ON, DEG)                # [ic, p, oh, on, d]
    Wd = np.transpose(Wd, (2, 0, 4, 1, 3))             # [oh, ic, d, p, on]
    Wd = np.ascontiguousarray(Wd.reshape(OH, NK, P, ON)).astype(bf16)

    bias = C[:, :, 0].sum(axis=0, dtype=np.float64).astype(np.float32)
    bias_rep = np.ascontiguousarray(np.broadcast_to(bias, (P, O_DIM)))

    in_maps = []
    for c in range(N_CORES):
        xt = np.ascontiguousarray(x[c * B_LOCAL:(c + 1) * B_LOCAL, :].T)
        in_maps.append({"xt": xt, "w": Wd, "bias": bias_rep})
    return in_maps


def kernel(x, cheby_coeffs):
    global last_results
    nc = _get_nc()
    in_maps = _prep_inputs(x, cheby_coeffs)
    last_results = run_bass_kernel_spmd(nc, in_maps,
                                        core_ids=list(range(N_CORES)))
    y = np.concatenate([r["y"] for r in last_results.results], axis=0)
    return y
